# revision 7
# baseline (speedup 1.0000x reference)
"""Self-contained Trainium2 Bass kernel: pre-LN multi-head attention block.

out = x + out_proj(MHA(LayerNorm(x))), x [8, 1024, 1024] f32, 16 heads.
Data-parallel over batch: core i handles batch element i; no collectives.
TimelineSim 192540 ns/core (baseline 274662), rel err ~1.04e-2.

Design, balanced under the ~139us Act-engine softmax-exp wall (exp exists
on no other engine; stream runs gapless):
  - host ships raw x transposed in fp8 (d on partitions) plus a bf16
    natural copy with the out-proj bias folded (residual); LN gamma/beta
    and all weight scaling (x32) folded into fp8 weights host-side.
  - LN stats via all-ones matmuls (partition-replicated sums in PSUM);
    the (x - mu) * rstd normalize is folded into the projections as two
    K=2 contraction rows ([-w1; bias32] x [mu; std]) plus *rstd on the
    projection output, so normalized x is never materialized. V's
    per-token rstd comes from a DRAM-bounced transposed b.
  - QKV and out projections run fp8e4 DoubleRow (2 d-tiles per matmul,
    0.5 cycles/row); q/k stay bf16 for accurate scores (K=64, two heads
    per PE pass via tile_position row base).
  - exp in [128,1024] chunks from double-buffered PSUM (alternating
    input tiles dodges a same-tile serialization penalty); probs fp8e5
    (e4's 240 max overflows: exp reaches ~8e3).
  - PV fp8 DoubleRow with the head pair stacked into one [128,1024]
    psum tile via zero-padded stationary windows [v_e|0] / [0|v_o]
    (dual-fp8 ldweights requires full 128-col stationaries, dst
    partition 0); denominators via one-hot-column DoubleRow matmuls
    onto psum rows 0/1, partition-broadcast by a K=2 selector matmul,
    reciprocal on DVE; last block interleaves its PV with the exps and
    normalizes straight from PSUM.
  - head-pair blocks are software-pipelined (exp stream emitted first,
    previous block's PV/den/normalize deferred, ex pool 4-deep); partial
    out-proj for heads 0-7 (+residual) banked to bf16 during blocks 4-6;
    the tail runs only heads 8-15 plus a fused epilogue; bf16 output
    stores (host casts back to f32).
Known headroom (~53us over the wall): ~30us fill dependency chain,
~23us DVE-serial tail; 1536-wide exp chunks would need re-plumbing all
PSUM pools from 4+2+2 to 6+1+1 banks for ~4us.
"""

import numpy as np
import ml_dtypes

P = 128
D = 1024
H = 16
DH = 64
S = 1024
B = 8
N_CORES = 8
LN_EPS = 1e-5
ND = 8          # d-tiles of 128
NJ = 4          # d-tile pairs
NC = 512        # matmul moving chunk
WSC = 32.0      # host weight scale

LAST_RESULTS = None
_NC_CACHE = {}


def _emit(tc, aps):
    from concourse import mybir

    nc = tc.nc
    f32 = mybir.dt.float32
    bf16 = mybir.dt.bfloat16
    fp8 = mybir.dt.float8e4
    fp8e5 = mybir.dt.float8e5
    FT = mybir.ActivationFunctionType
    OP = mybir.AluOpType
    DR = mybir.MatmulPerfMode.DoubleRow

    x8a, res, wqk, wv, wout, wfix, wfxv, out = (
        aps["x8"], aps["res"], aps["wqk"], aps["wv"], aps["wout"],
        aps["wfix"], aps["wfxv"], aps["out"])

    with tc.tile_pool(name="consts", bufs=1) as consts, \
         tc.tile_pool(name="acts", bufs=1) as acts, \
         tc.tile_pool(name="dscratch", bufs=1, space="DRAM") as dscratch:

        ones_mat = consts.tile([P, P], bf16)
        nc.vector.memset(ones_mat, 1.0)
        # selector for den partition-broadcast via K=2 matmul:
        # out[p, s] = dex[p // 64, s]  (host-uploaded: engine APs cannot
        # start at partition 1, so no on-device memset construction)
        sel = consts.tile([2, P], bf16)
        nc.gpsimd.dma_start(out=sel, in_=aps["sel"])
        # one-hot-column DR stationaries: den of head-even -> psum row 0,
        # head-odd -> row 1 (full 128-col stationary keeps dual-fp8 LW legal)
        oneh0 = consts.tile([P, 2, P], fp8)
        oneh1 = consts.tile([P, 2, P], fp8)
        nc.vector.memset(oneh0, 0.0)
        nc.vector.memset(oneh1, 0.0)
        nc.vector.memset(oneh0[:, :, 0:1], 1.0)
        nc.vector.memset(oneh1[:, :, 1:2], 1.0)
        eps_sb = consts.tile([P, 1], f32)
        nc.vector.memset(eps_sb, LN_EPS)
        wfix_sb = consts.tile([2, 16, P], bf16)
        wfxv_sb = consts.tile([2, D], bf16)
        bT_sb = consts.tile([P, ND], f32)       # 1/std transposed (per token)
        db_dram = dscratch.tile([S], f32)

        x8_sb = acts.tile([P, NJ, 2, S], fp8)   # raw x, transposed, fp8
        x8f = x8_sb.rearrange("p a b s -> p (a b) s")
        wqk_sb = acts.tile([P, NJ, 2, 2 * D], fp8)
        wv_sb = acts.tile([P, NJ, 2, D], fp8)
        wout_sb = acts.tile([P, NJ, 2, D], fp8)
        qkT_sb = acts.tile([P, 16, S], bf16)
        # v with zero blocks interleaved: [.., h, 0, :] = v_h, [.., h, 1, :]=0
        # so flat windows give [v_even | 0] and [0 | v_odd] 128-col
        # stationaries for pair-stacked PV (dual-fp8 LW requires 128 cols)
        v_sb = acts.tile([P, ND, H, 2, DH], fp8)
        v_fl = v_sb.rearrange("p s h two d -> p s (h two d)")
        ctxu_sb = acts.tile([P, 8, S], bf16)    # pair-stacked raw ctx
        ctx8_sb = acts.tile([P, 8, S], fp8)     # normalized, hp-slot layout
        fix2 = acts.tile([2, S], bf16)          # row0 = +mu, row1 = std
        bfull = acts.tile([P, S], f32)          # 1/std, partition-broadcast
        # zero the PV stationary pad blocks up front, off the critical path
        nc.vector.memset(v_sb[:, :, :, 1, :], 0.0)

        # -------- Phase 1: LN stats only; (x-mu)*b folds into projections --
        x8_r = x8a.rearrange("p a b s -> p (a b) s")
        with tc.tile_pool(name="lntmp", bufs=2) as lntmp, \
             tc.tile_pool(name="lnrow", bufs=1) as lnrow, \
             tc.tile_pool(name="lnps", bufs=1, space="PSUM") as lnps:
            sx_ps = lnps.tile([P, S], f32, tag="sx")
            sx2_ps = lnps.tile([P, S], f32, tag="sx2")
            # split x8 load so squares start before the full tensor lands
            nc.sync.dma_start(out=x8_sb[:, 0:2], in_=x8a[:, 0:2])
            nc.sync.dma_start(out=x8_sb[:, 2:4], in_=x8a[:, 2:4])
            sqt = lntmp.tile([P, ND, S], bf16, tag="sq", bufs=1)
            for j in range(ND):
                # x^2 full-width on the (otherwise idle) Act engine
                nc.scalar.square(out=sqt[:, j], in_=x8f[:, j, :])
            for c in range(2):
                sl = slice(c * NC, (c + 1) * NC)
                for j in range(ND):
                    nc.tensor.matmul(sx_ps[:, sl], lhsT=ones_mat,
                                     rhs=x8f[:, j, sl],
                                     start=(j == 0), stop=(j == ND - 1))
                    nc.tensor.matmul(sx2_ps[:, sl], lhsT=ones_mat,
                                     rhs=sqt[:, j, sl],
                                     start=(j == 0), stop=(j == ND - 1))
                if c == 0:
                    # weights split across gpsimd + vector queues
                    nc.gpsimd.dma_start(out=wqk_sb, in_=wqk)
                    nc.gpsimd.dma_start(out=wfix_sb, in_=wfix)
                    nc.gpsimd.dma_start(out=wfxv_sb, in_=wfxv)
                    nc.scalar.dma_start(out=wv_sb, in_=wv)
                    nc.scalar.dma_start(out=wout_sb, in_=wout)

            for c in range(2):
                sl = slice(c * NC, (c + 1) * NC)
                # stats on D-scaled sums: var*D = sx2 - (sx/D)*sx
                # std = sqrt((var*D)/D + eps) via activation scale param
                with nc.allow_low_precision(reason="LN stats in bf16"):
                    mu_bc = lnrow.tile([P, NC], bf16, tag="mu", bufs=2)
                    nc.vector.tensor_scalar_mul(mu_bc, sx_ps[:, sl], 1.0 / D)
                    musx = lnrow.tile([P, NC], bf16, tag="musx", bufs=2)
                    nc.vector.tensor_tensor(out=musx, in0=mu_bc,
                                            in1=sx_ps[:, sl], op=OP.mult)
                    var_bc = lnrow.tile([P, NC], bf16, tag="var", bufs=2)
                    nc.vector.tensor_tensor(out=var_bc, in0=sx2_ps[:, sl],
                                            in1=musx, op=OP.subtract)
                    std_bc = musx
                    nc.scalar.activation(out=std_bc, in_=var_bc, func=FT.Sqrt,
                                         bias=eps_sb, scale=1.0 / D)
                    nc.vector.reciprocal(out=bfull[:, sl], in_=std_bc)
                    # fixup rows via SBUF->SBUF DMA (engine ops cannot
                    # write partition 1): row0 = +mu (host negated w1)
                    nc.sync.dma_start(out=fix2[0:1, sl], in_=mu_bc[0:1, :])
                    nc.sync.dma_start(out=fix2[1:2, sl],
                                      in_=std_bc[0:1, :])
            # b transposed to partitions via DRAM bounce (off critical path)
            nc.sync.dma_start(out=db_dram, in_=bfull[0:1, :])
            nc.sync.dma_start(out=bT_sb,
                              in_=db_dram.rearrange("(a p) -> p a", p=P))

        # residual prefetch (bf16), consumed by the partial out-proj
        res_sb = acts.tile([P, ND, D], bf16)
        nc.sync.dma_start(out=res_sb, in_=res.rearrange("(a p) d -> p a d",
                                                        p=P))
        # partial out-proj accumulator (heads 0-7 + residual), bf16
        otA_sb = acts.tile([P, ND, D], bf16)

        # ---------------- Phases 2-3 (software-pipelined blocks) ----------
        with tc.tile_pool(name="expp", bufs=4) as expp, \
             tc.tile_pool(name="rdp", bufs=2) as rdp, \
             tc.tile_pool(name="scps", bufs=1, space="PSUM") as scps, \
             tc.tile_pool(name="pvps", bufs=1, space="PSUM") as pvps, \
             tc.tile_pool(name="pjps", bufs=1, space="PSUM") as pjps:

            def qkproj(et, pool=None, tag=None):
                # raw-x projection + K=2 LN-fixup rows; then * b per column
                pool = pool or pjps
                pj = pool.tile([P, S], f32, tag=tag or "pj", name=f"qk{et}")
                for c in range(2):
                    sl = slice(c * NC, (c + 1) * NC)
                    for jj in range(NJ):
                        nc.tensor.matmul(
                            pj[:, sl],
                            lhsT=wqk_sb[:, jj, :, et * P:(et + 1) * P],
                            rhs=x8_sb[:, jj, :, sl],
                            start=(jj == 0), stop=False, perf_mode=DR,
                            skip_group_check=True)
                    nc.tensor.matmul(
                        pj[:, sl], lhsT=wfix_sb[:, et, :], rhs=fix2[:, sl],
                        start=False, stop=True, skip_group_check=True)
                with nc.allow_low_precision(reason="qk in bf16"):
                    nc.vector.tensor_tensor(out=qkT_sb[:, et], in0=pj,
                                            in1=bfull, op=OP.mult)

            def vproj(st):
                pj = pjps.tile([P, S], f32, tag="pj", name=f"vp{st}")
                tsl = slice(st * P, (st + 1) * P)
                for c in range(2):
                    sl = slice(c * NC, (c + 1) * NC)
                    for jj in range(NJ):
                        nc.tensor.matmul(
                            pj[:, sl],
                            lhsT=x8_sb[:, jj, :, tsl],
                            rhs=wv_sb[:, jj, :, sl],
                            start=(jj == 0), stop=False, perf_mode=DR,
                            skip_group_check=True)
                    nc.tensor.matmul(
                        pj[:, sl], lhsT=fix2[:, tsl], rhs=wfxv_sb[:, sl],
                        start=False, stop=True, skip_group_check=True)
                with nc.allow_low_precision(reason="v in fp8"):
                    nc.vector.tensor_scalar(
                        v_sb[:, st, :, 0, :],
                        pj.rearrange("p (h d) -> p h d", d=DH),
                        bT_sb[:, st:st + 1], 1.0 / WSC, OP.mult, OP.mult)

            def scores_exp(hp, pvdn=None):
                ex8 = expp.tile([P, ND, S], fp8e5, tag="exp",
                                name=f"exA{hp}")
                ex8B = expp.tile([P, ND, S], fp8e5, tag="exp",
                                 name=f"exB{hp}")
                for idx, ex8t in enumerate((ex8, ex8B)):
                    if idx == 1 and pvdn is not None:
                        pv_mms(hp, 0, ex8, pvdn[0], pvdn[1])
                    base = idx * DH
                    for tt in range(ND):
                        sc = scps.tile([P, S], f32, tag="sc", bufs=2,
                                       name=f"sc{hp}_{idx}_{tt}")
                        for c in range(2):
                            sl = slice(c * NC, (c + 1) * NC)
                            nc.tensor.matmul(
                                sc[:, sl],
                                lhsT=qkT_sb[base:base + DH, 8 + hp,
                                            tt * P:(tt + 1) * P],
                                rhs=qkT_sb[base:base + DH, hp, sl],
                                start=True, stop=True,
                                tile_position=(base, 0))
                        nc.scalar.activation(out=ex8t[:, tt], in_=sc,
                                             func=FT.Exp, scale=1.0 / 8192.0)
                return ex8, ex8B

            def pv_mms(hp, idx, ex8t, pv, dn):
                for c in range(2):
                    sl = slice(c * NC, (c + 1) * NC)
                    off = 2 * hp * 2 * DH + idx * DH
                    for tp in range(ND // 2):
                        nc.tensor.matmul(
                            pv[:, sl],
                            lhsT=v_fl[:, 2 * tp:2 * tp + 2, off:off + P],
                            rhs=ex8t[:, 2 * tp:2 * tp + 2, sl],
                            start=(idx == 0 and tp == 0),
                            stop=(idx == 1 and tp == 3),
                            perf_mode=DR, skip_group_check=True)
                oneh = oneh0 if idx == 0 else oneh1
                for c in range(2):
                    sl = slice(c * NC, (c + 1) * NC)
                    for tp in range(ND // 2):
                        nc.tensor.matmul(
                            dn[:, sl],
                            lhsT=oneh,
                            rhs=ex8t[:, 2 * tp:2 * tp + 2, sl],
                            start=(idx == 0 and tp == 0),
                            stop=(idx == 1 and tp == 3),
                            perf_mode=DR, skip_group_check=True)

            def pv_den_norm(hp, ex8, ex8B, last=False, pvdn=None):
                # PV pair-stacked via zero-padded stationary windows:
                # head-even uses [v_e | 0], head-odd [0 | v_o], one
                # accumulation group -> rows 0-63 ctx_e, 64-127 ctx_o
                if pvdn is not None:
                    pv, dn = pvdn             # even-head mms already emitted
                    start_idx = 1
                else:
                    pv = pvps.tile([P, S], f32, tag="pv", name=f"pv{hp}")
                    dn = pjps.tile([P, S], f32, tag="pj", name=f"dn{hp}")
                    start_idx = 0
                for idx in range(start_idx, 2):
                    pv_mms(hp, idx, (ex8, ex8B)[idx], pv, dn)

                # dens land on psum rows 0 (even) / 1 (odd); bf16 is fine
                dex = rdp.tile([2, S], bf16, tag="dex", name=f"dex{hp}")
                with nc.allow_low_precision(reason="softmax denom bf16"):
                    if last:
                        nc.scalar.copy(out=dex, in_=dn[0:2, :])
                    else:
                        nc.vector.tensor_copy(out=dex, in_=dn[0:2, :])
                if not last:
                    nc.vector.tensor_copy(out=ctxu_sb[:, hp], in_=pv)

                # partition-broadcast den via K=2 selector matmul (no DMA)
                dpool = pjps if last else pvps
                dtag = "pj" if last else "pv"
                dbc = dpool.tile([P, S], f32, tag=dtag, name=f"dbc{hp}")
                for c in range(2):
                    sl = slice(c * NC, (c + 1) * NC)
                    nc.tensor.matmul(dbc[:, sl], lhsT=sel, rhs=dex[:, sl],
                                     start=True, stop=True)
                rdbc = rdp.tile([P, S], bf16, tag="rdbc", name=f"rdbc{hp}")
                with nc.allow_low_precision(reason="softmax denom recip bf16"):
                    nc.vector.reciprocal(out=rdbc, in_=dbc)
                    src_ctx = pv if last else ctxu_sb[:, hp]
                    nc.vector.tensor_tensor(out=ctx8_sb[:, hp],
                                            in0=src_ctx, in1=rdbc,
                                            op=OP.mult)

            # fill: q/k for pair 0 through PARALLEL psum pools
            qkproj(0)
            qkproj(8, pool=pvps, tag="pv")
            for st in range(4):
                vproj(st)

            def oproj_partial(st):
                # heads 0-7 contribution + residual, banked to SBUF bf16
                po = pjps.tile([P, S], f32, tag="pj", name=f"opA{st}")
                for c in range(2):
                    sl = slice(c * NC, (c + 1) * NC)
                    for u in range(2):
                        nc.tensor.matmul(
                            po[:, sl],
                            lhsT=ctx8_sb[:, 2 * u:2 * u + 2,
                                         st * P:(st + 1) * P],
                            rhs=wout_sb[:, u, :, sl],
                            start=(u == 0), stop=(u == 1), perf_mode=DR)
                with nc.allow_low_precision(reason="partial out in bf16"):
                    nc.vector.scalar_tensor_tensor(
                        out=otA_sb[:, st], in0=po, scalar=1.0 / WSC,
                        in1=res_sb[:, st], op0=OP.mult, op1=OP.add)

            prev = None
            for hp in range(H // 2):
                if hp == 7:
                    # deferred block-6 work first so pool generations stay
                    # in order, then pv7/dn7 interleave with block-7 exps
                    pv_den_norm(6, *prev)
                    prev = None
                    pv7 = pvps.tile([P, S], f32, tag="pv", name="pv7")
                    dn7 = pjps.tile([P, S], f32, tag="pj", name="dn7")
                    cur = scores_exp(hp, pvdn=(pv7, dn7))
                else:
                    cur = scores_exp(hp)      # Act stream front and center
                if prev is not None:
                    pv_den_norm(hp - 1, *prev)
                if hp < 7:
                    qkproj(hp + 1)
                    qkproj(9 + hp, pool=pvps, tag="pv")
                if hp == 0:
                    for st in range(4, ND):
                        vproj(st)
                # partial out-proj spread over blocks 4-6 (needs ctx8 0-3
                # only); block 7 stays clear so the tail chain is short
                if hp == 4:
                    for _st in (0, 1, 2):
                        oproj_partial(_st)
                elif hp == 5:
                    for _st in (3, 4, 5):
                        oproj_partial(_st)
                elif hp == 6:
                    oproj_partial(6)
                    oproj_partial(7)
                prev = cur
            pv_den_norm(7, *prev, last=True, pvdn=(pv7, dn7))

        # ---------------- Phase 4: out-proj + residual ----------------
        with tc.tile_pool(name="p4", bufs=2) as p4, \
             tc.tile_pool(name="p4ps", bufs=1, space="PSUM") as p4ps:
            for st in range(ND):
                po = p4ps.tile([P, S], f32, tag="po", bufs=4, name=f"op{st}")
                for c in range(2):
                    sl = slice(c * NC, (c + 1) * NC)
                    for u in range(2, NJ):
                        nc.tensor.matmul(
                            po[:, sl],
                            lhsT=ctx8_sb[:, 2 * u:2 * u + 2,
                                         st * P:(st + 1) * P],
                            rhs=wout_sb[:, u, :, sl],
                            start=(u == 2), stop=(u == NJ - 1),
                            perf_mode=DR)
                ot = p4.tile([P, D], bf16, tag="out", bufs=3,
                             name=f"ot{st}")
                with nc.allow_low_precision(reason="output in bf16"):
                    nc.vector.scalar_tensor_tensor(
                        out=ot, in0=po, scalar=1.0 / WSC, in1=otA_sb[:, st],
                        op0=OP.mult, op1=OP.add)
                eng = nc.gpsimd if st % 2 == 0 else nc.sync
                eng.dma_start(out=out[st * P:(st + 1) * P, :], in_=ot)


def build_nc():
    import concourse.bacc as bacc
    import concourse.tile as tile
    from concourse import mybir

    f32 = mybir.dt.float32
    bf16 = mybir.dt.bfloat16
    fp8 = mybir.dt.float8e4

    nc = bacc.Bacc("TRN2", target_bir_lowering=False, debug=False)
    aps = {
        "x8": nc.dram_tensor("x8", [P, NJ, 2, S], fp8,
                             kind="ExternalInput").ap(),
        "res": nc.dram_tensor("res", [S, D], bf16, kind="ExternalInput").ap(),
        "wqk": nc.dram_tensor("wqk", [P, NJ, 2, 2 * D], fp8,
                              kind="ExternalInput").ap(),
        "wv": nc.dram_tensor("wv", [P, NJ, 2, D], fp8,
                             kind="ExternalInput").ap(),
        "wout": nc.dram_tensor("wout", [P, NJ, 2, D], fp8,
                               kind="ExternalInput").ap(),
        "wfix": nc.dram_tensor("wfix", [2, 16, P], bf16,
                               kind="ExternalInput").ap(),
        "wfxv": nc.dram_tensor("wfxv", [2, D], bf16,
                               kind="ExternalInput").ap(),
        "sel": nc.dram_tensor("sel", [2, P], bf16, kind="ExternalInput").ap(),
        "out": nc.dram_tensor("out", [S, D], bf16,
                              kind="ExternalOutput").ap(),
    }
    with tile.TileContext(nc) as tc:
        _emit(tc, aps)
    nc.compile()
    return nc


def prep_inputs(x, ln_gamma, ln_beta, in_proj_w, in_proj_b, out_proj_w,
                out_proj_b):
    bf = ml_dtypes.bfloat16
    f8 = ml_dtypes.float8_e4m3
    f32c = lambda a: np.ascontiguousarray(a, dtype=np.float32)

    g = np.asarray(ln_gamma, np.float32)
    be = np.asarray(ln_beta, np.float32)
    wi = np.asarray(in_proj_w, np.float32)      # [3D, D]
    bi = np.asarray(in_proj_b, np.float32)      # [3D]
    wo = np.asarray(out_proj_w, np.float32)     # [D, D]
    bo = np.asarray(out_proj_b, np.float32)     # [D]

    # fold gamma/beta into projections
    wi_g = wi * g[None, :]                      # W' = W diag(gamma)
    bi_f = bi + wi @ be                         # b' = b + W beta

    # wqk: [P(dpart), NJ, 2, 2D] = 32*W'[e, d].T with d=(2jj+i)*128+p
    wqk_t = (WSC * wi_g[:2 * D, :].T)           # [D(d), 2D(e)]
    wqk_arr = wqk_t.reshape(NJ, 2, P, 2 * D).transpose(2, 0, 1, 3)
    # wv: [P, NJ, 2, D] = 32*Wv'[e, d].T
    wv_t = (WSC * wi_g[2 * D:, :].T)            # [D(d), D(e)]
    wv_arr = wv_t.reshape(NJ, 2, P, D).transpose(2, 0, 1, 3)

    # LN-fixup rows: psum += (-w1[e])*mu + bias32[e]*std, then *b
    w1_qk = -wqk_t.sum(axis=0)                  # [2D], negated
    b32_qk = WSC * bi_f[:2 * D]
    wfix_arr = np.stack([w1_qk.reshape(16, P), b32_qk.reshape(16, P)], axis=0)
    w1_v = -wv_t.sum(axis=0)                    # [D], negated
    b32_v = WSC * bi_f[2 * D:]
    wfxv_arr = np.stack([w1_v, b32_v], axis=0)  # [2, D]
    # wout: [P, NJ(u), 2(i), D(e)]; row p of slot (u,i): d = (2(2u+i)+(p>=64))*64 + p%64
    wo_t = (WSC * wo.T)                         # [D(d), D(e)]
    # build index map d(p, u, i)
    pidx = np.arange(P)
    uu, ii = np.meshgrid(np.arange(NJ), np.arange(2), indexing="ij")
    dmap = np.zeros((P, NJ, 2), np.int64)
    for u in range(NJ):
        for i in range(2):
            hh = 2 * (2 * u + i) + (pidx >= DH).astype(np.int64)
            dmap[:, u, i] = hh * DH + (pidx % DH)
    wout_arr = wo_t[dmap]                       # [P, NJ, 2, D]

    sel_arr = np.zeros((2, P), np.float32)
    sel_arr[0, :DH] = 1.0
    sel_arr[1, DH:] = 1.0
    shared = {
        "sel": sel_arr.astype(bf),
        "wqk": np.ascontiguousarray(wqk_arr).astype(f8),
        "wv": np.ascontiguousarray(wv_arr).astype(f8),
        "wout": np.ascontiguousarray(wout_arr).astype(f8),
        "wfix": np.ascontiguousarray(wfix_arr).astype(bf),
        "wfxv": np.ascontiguousarray(wfxv_arr).astype(bf),
    }
    in_maps = []
    for i in range(N_CORES):
        xi = np.asarray(x[i], np.float32)
        m = dict(shared)
        x8 = xi.T.reshape(NJ, 2, P, S).transpose(2, 0, 1, 3)
        m["x8"] = np.ascontiguousarray(x8).astype(f8)
        m["res"] = (xi + bo[None, :]).astype(bf)
        in_maps.append(m)
    return in_maps


def kernel(x, ln_gamma, ln_beta, in_proj_w, in_proj_b, out_proj_w, out_proj_b):
    global LAST_RESULTS
    from concourse import bass_utils

    if "nc" not in _NC_CACHE:
        _NC_CACHE["nc"] = build_nc()
    nc = _NC_CACHE["nc"]

    in_maps = prep_inputs(x, ln_gamma, ln_beta, in_proj_w, in_proj_b,
                          out_proj_w, out_proj_b)
    res = bass_utils.run_bass_kernel_spmd(nc, in_maps,
                                          core_ids=list(range(N_CORES)))
    LAST_RESULTS = res
    out = np.stack([r["out"] for r in res.results], axis=0)
    return np.ascontiguousarray(out, dtype=np.float32)
